# revision 34
# baseline (speedup 1.0000x reference)
"""Cross-attention Trainium2 kernel (8 NeuronCores).

Sharding: batch (2) x head-groups (4 groups of 4 heads) = 8 shards.
Each core computes q/k/v projections for its 4 heads (256 cols of
Wq/Wk/Wv), attention for those heads, and a partial out-projection
through its 256 rows of Wo.  The host sums the 4 partial outputs per
batch (the reduction of the head-parallel out_proj) and adds the
bv @ Wo + bo correction, which commutes exactly through the softmax
average.

Layout strategy on-core:
  - x/ctx are PE-transposed (xT: [d, s]) so projections contract d on
    partitions; projections emit qT/kT ([head_dim, s]) and v (natural).
  - scores are computed transposed (ST = k @ qT -> [sk, sq]) so the
    exp'd tiles feed the attention matmul directly as the stationary
    operand; a ones-column in v gives the softmax denominator for free.
  - matmuls run as float32r (full-rate fp32); P/v in fp16.
"""

import numpy as np

import concourse.bass as bass
import concourse.mybir as mybir
import concourse.tile as tile
from concourse import bacc

B, SQ, SK, D, H, HS = 2, 2048, 2048, 1024, 16, 64
SCALE = HS ** -0.5
NCORES = 8
HG = 4            # heads per core
DG = HG * HS      # 256 projection cols per core

F32 = mybir.dt.float32
F32R = mybir.dt.float32r
F16 = mybir.dt.float16


def build_program(fast_mm: bool = True, pipeline: bool = True, loop_iters: int = 0):
    """Build the per-core SPMD Bass program."""
    MMDT = F32R if fast_mm else F32

    nc = bacc.Bacc(None, target_bir_lowering=False, debug=False,
                   num_devices=NCORES)
    x_d = nc.dram_tensor("x", [SQ, D], MMDT, kind="ExternalInput")
    c_d = nc.dram_tensor("ctx", [SK, D], MMDT, kind="ExternalInput")
    wq_d = nc.dram_tensor("wq", [D, DG], MMDT, kind="ExternalInput")
    wk_d = nc.dram_tensor("wk", [D, DG], MMDT, kind="ExternalInput")
    wv_d = nc.dram_tensor("wv", [D, DG], MMDT, kind="ExternalInput")
    wo_d = nc.dram_tensor("wo", [DG, D], F16, kind="ExternalInput")
    bq_d = nc.dram_tensor("bq", [DG], F32, kind="ExternalInput")
    bk_d = nc.dram_tensor("bk", [DG], F32, kind="ExternalInput")
    i_d = nc.dram_tensor("ident", [128, 128], MMDT, kind="ExternalInput")
    i16_d = nc.dram_tensor("ident16", [128, 128], F16, kind="ExternalInput")
    out_d = nc.dram_tensor("out", [SQ, D], F32, kind="ExternalOutput")

    with tile.TileContext(nc) as tc:
        with (
            tc.tile_pool(name="const", bufs=1) as cp,
            tc.tile_pool(name="persist", bufs=1) as psb,
            tc.tile_pool(name="xw", bufs=5) as xwp,
            tc.tile_pool(name="xtw", bufs=1) as xtwp,
            tc.tile_pool(name="expp", bufs=22) as ep,
            tc.tile_pool(name="fin", bufs=4) as fpool,
            tc.tile_pool(name="outp", bufs=3) as opool,
            tc.tile_pool(name="pp", bufs=2, space="PSUM") as pp,
            tc.tile_pool(name="stp", bufs=2, space="PSUM") as stp,
            tc.tile_pool(name="atp", bufs=2, space="PSUM") as atp,
        ):
            import contextlib
            loop_ctx = tc.For_i(0, loop_iters, 1) if loop_iters else contextlib.nullcontext()
            loop_ctx.__enter__()
            ident = cp.tile([128, 128], MMDT)
            nc.sync.dma_start(out=ident, in_=i_d[:])
            ident16 = cp.tile([128, 128], F16, tag="ident16")
            nc.sync.dma_start(out=ident16, in_=i16_d[:])

            wq_sb = cp.tile([128, 8, DG], MMDT, tag="wq")
            wk_sb = cp.tile([128, 8, DG], MMDT, tag="wk")
            wv_sb = cp.tile([128, 8, DG], MMDT, tag="wv")
            wo_sb = cp.tile([128, 2, D], F16, tag="wo")
            bq_sb = cp.tile([128, 2], F32, tag="bq")
            bk_sb = cp.tile([128, 2], F32, tag="bk")
            def load_weights_qx():
                nc.sync.dma_start(out=wq_sb, in_=wq_d[:].rearrange("(c p) n -> p c n", p=128))
                nc.sync.dma_start(out=bq_sb, in_=bq_d[:].rearrange("(c p) -> p c", p=128))

            def load_weights_kv():
                nc.sync.dma_start(out=wk_sb, in_=wk_d[:].rearrange("(c p) n -> p c n", p=128))
                nc.sync.dma_start(out=wv_sb, in_=wv_d[:].rearrange("(c p) n -> p c n", p=128))
                nc.sync.dma_start(out=bk_sb, in_=bk_d[:].rearrange("(c p) -> p c", p=128))

            def load_weights_o():
                nc.sync.dma_start(out=wo_sb, in_=wo_d[:].rearrange("(c p) n -> p c n", p=128))

            # persistent activations, split per producing window so the
            # scheduler's dependencies stay fine-grained
            qTs = [psb.tile([128, 2, 512], MMDT, tag=f"qT{w}", name=f"qT{w}") for w in range(4)]
            kTs = [psb.tile([128, 2, 512], MMDT, tag=f"kT{w}", name=f"kT{w}") for w in range(4)]
            vAs = [psb.tile([128, 4, HG, 68], F16, tag=f"vA{w}", name=f"vA{w}") for w in range(4)]
            aTs = [psb.tile([128, 2, 128], F16, tag=f"aT{s}", name=f"aT{s}") for s in range(16)]

            for w in range(4):
                nc.vector.memset(vAs[w][:], 1.0)

            def proj_window(src_d, dst_T, bias_sb, w_sb, with_v, w):
                xts = []
                for i in range(4):
                    xt = xwp.tile([128, D], MMDT, tag="xw")
                    r0 = (w * 4 + i) * 128
                    nc.sync.dma_start(out=xt, in_=src_d[r0:r0 + 128, :])
                    xts.append(xt)
                xtw = xtwp.tile([128, 8, 512], MMDT, tag="xtw")
                for dc in range(8):
                    pt = pp.tile([128, 512], MMDT, tag="pp")
                    for i in range(4):
                        nc.tensor.transpose(
                            (pt[:, i * 128:(i + 1) * 128]),
                            (xts[i][:, dc * 128:(dc + 1) * 128]),
                            (ident),
                        )
                    nc.vector.tensor_copy(xtw[:, dc, :], pt)
                for c in range(2):
                    pq = pp.tile([128, 512], F32, tag="pp")
                    for dc in range(8):
                        nc.tensor.matmul(
                            pq,
                            (w_sb[:, dc, c * 128:(c + 1) * 128]),
                            (xtw[:, dc, :]),
                            start=(dc == 0), stop=(dc == 7),
                        )
                    nc.vector.tensor_scalar_add(
                        dst_T[w][:, c, :], pq, bias_sb[:, c:c + 1])
                if with_v:
                    for s in range(4):
                        # attention psum pool is idle during projections
                        pv = atp.tile([128, 512], F32, tag="at")
                        for dc in range(8):
                            nc.tensor.matmul(
                                pv[:, :DG],
                                (xtw[:, dc, s * 128:(s + 1) * 128]),
                                (wv_sb[:, dc, :]),
                                start=(dc == 0), stop=(dc == 7),
                            )
                        nc.vector.tensor_copy(
                            vAs[w][:, s, :, 0:64],
                            pv[:, :DG].rearrange("p (h e) -> p h e", e=64),
                        )

            def proj_x(w):
                proj_window(x_d, qTs, bq_sb, wq_sb, False, w)

            def proj_ctx(w):
                proj_window(c_d, kTs, bk_sb, wk_sb, True, w)

            # attention per head / sq-window of 1024, software-pipelined:
            # window w's attn-matmuls are emitted after window w+1's
            # scores+exp so ACT (exp) is never starved.
            def emit_scores_exp(h, sqw, skcs):
                p0 = 64 * (h % 2)
                t = h // 2
                exs = []
                for skc in skcs:
                    st = stp.tile([128, 1024], F32, tag="st")
                    for half in range(2):
                        qw = sqw * 2 + half
                        nc.tensor.matmul(
                            st[:, half * 512:(half + 1) * 512],
                            (kTs[skc // 4][p0:p0 + 64, t,
                                             (skc % 4) * 128:(skc % 4 + 1) * 128]),
                            (qTs[qw][p0:p0 + 64, t, :]),
                            start=True, stop=True,
                        )
                    ex = ep.tile([128, 1024], F16, tag="ex")
                    nc.scalar.activation(
                        ex, st, mybir.ActivationFunctionType.Exp,
                        scale=SCALE)
                    exs.append(ex)
                return exs

            def emit_attnv_fin(h, sqw, exs):
                p0 = 64 * (h % 2)
                t = h // 2
                # attn accumulation: one psum bank per sq-chunk j
                for j in range(8):
                    at = atp.tile([128, 512], F32, tag="at")
                    for skc in range(16):
                        nc.tensor.matmul(
                            at[:, 0:68],
                            exs[skc][:, j * 128:(j + 1) * 128],
                            vAs[skc // 4][:, skc % 4, h, :],
                            start=(skc == 0), stop=(skc == 15),
                        )
                    # normalize + transpose into aT
                    rc = fpool.tile([128, 1], F32, tag="rc")
                    nc.vector.reciprocal(rc, at[:, 64:65])
                    ad = fpool.tile([128, 64], F16, tag="ad")
                    nc.vector.tensor_scalar_mul(ad, at[:, 0:64], rc)
                    pt2 = pp.tile([128, 128], F16, tag="pp")
                    nc.tensor.transpose(pt2[p0:p0 + 64, :], ad, ident16)
                    nc.vector.tensor_copy(
                        aTs[sqw * 8 + j][p0:p0 + 64, t, :],
                        pt2[p0:p0 + 64, :])

            # out projection for a range of sq chunks (partial out: this
            # core's 256 attn cols)
            def emit_out_proj(sqcs):
                for sqc in sqcs:
                    ot = opool.tile([128, D], F32, tag="ot")
                    for n2 in range(2):
                        po = pp.tile([128, 512], F32, tag="pp")
                        for kc in range(2):
                            nc.tensor.matmul(
                                po,
                                (aTs[sqc][:, kc, :]),
                                (wo_sb[:, kc, n2 * 512:(n2 + 1) * 512]),
                                start=(kc == 0), stop=(kc == 1),
                            )
                        nc.vector.tensor_copy(ot[:, n2 * 512:(n2 + 1) * 512], po)
                    nc.sync.dma_start(
                        out=out_d[sqc * 128:(sqc + 1) * 128, :], in_=ot)

            if pipeline:
                # interleave projections with the first attention window's
                # scores so ACT (exp) starts as early as possible
                load_weights_qx()
                proj_x(0)
                proj_x(1)
                load_weights_kv()
                proj_ctx(0)
                exs0 = emit_scores_exp(0, 0, range(0, 4))
                proj_ctx(1)
                exs0 += emit_scores_exp(0, 0, range(4, 8))
                proj_x(2)
                proj_ctx(2)
                exs0 += emit_scores_exp(0, 0, range(8, 12))
                proj_x(3)
                proj_ctx(3)
                load_weights_o()
                exs0 += emit_scores_exp(0, 0, range(12, 16))
                pending = (0, 0, exs0)
                # out_proj chunks woven into the later (ACT-bound) windows
                op_after = {3: range(0, 2), 4: range(2, 4), 5: range(4, 6),
                            6: range(6, 8)}
                windows = [(h, sqw) for sqw in range(2) for h in range(HG)]
                for i, (h, sqw) in enumerate(windows[1:]):
                    exs = emit_scores_exp(h, sqw, range(0, 4))
                    emit_attnv_fin(*pending)
                    if i in op_after:
                        emit_out_proj(op_after[i])
                    pending = (h, sqw, exs)
                    exs += emit_scores_exp(h, sqw, range(4, 16))
                emit_attnv_fin(*pending)
                emit_out_proj(range(8, 16))
            else:
                load_weights_qx()
                load_weights_kv()
                load_weights_o()
                for w in range(4):
                    proj_x(w)
                for w in range(4):
                    proj_ctx(w)
                for h in range(HG):
                    for sqw in range(2):
                        exs = emit_scores_exp(h, sqw, range(16))
                        emit_attnv_fin(h, sqw, exs)
                emit_out_proj(range(16))
            loop_ctx.__exit__(None, None, None)

    nc.compile()
    return nc


_NC = None


def _program():
    global _NC
    if _NC is None:
        _NC = build_program()
    return _NC


def _f32(a):
    return np.ascontiguousarray(np.asarray(a, dtype=np.float32))


def kernel(inputs, context, Wq, bq, Wk, bk, Wv, bv, Wo, bo):
    from concourse.bass_utils import run_bass_kernel_spmd

    inputs = _f32(inputs)
    context = _f32(context)
    Wq, bq, Wk, bk = _f32(Wq), _f32(bq), _f32(Wk), _f32(bk)
    Wv, bv, Wo, bo = _f32(Wv), _f32(bv), _f32(Wo), _f32(bo)

    nc = _program()
    in_maps = []
    for core in range(NCORES):
        b, g = core // HG, core % HG
        sl = slice(DG * g, DG * (g + 1))
        in_maps.append({
            "x": inputs[b],
            "ctx": context[b],
            "wq": _f32(Wq[:, sl]),
            "wk": _f32(Wk[:, sl]),
            "wv": _f32(Wv[:, sl]),
            "wo": np.ascontiguousarray(Wo[sl, :].astype(np.float16)),
            "bq": _f32(bq[sl]),
            "bk": _f32(bk[sl]),
            "ident": np.eye(128, dtype=np.float32),
            "ident16": np.eye(128, dtype=np.float16),
        })
    res = run_bass_kernel_spmd(nc, in_maps, list(range(NCORES)))
    outs = [res.results[i]["out"] for i in range(NCORES)]
    corr = (bv.astype(np.float64) @ Wo.astype(np.float64)
            + bo.astype(np.float64)).astype(np.float32)
    full = np.stack([
        outs[0] + outs[1] + outs[2] + outs[3],
        outs[4] + outs[5] + outs[6] + outs[7],
    ]) + corr
    return full.astype(np.float32)
